# revision 2
# baseline (speedup 1.0000x reference)
"""AdaptiveLIF spiking-neuron kernel for 8 TRN2 NeuronCores.

Reference recurrence (per element, over T steps):
    v = v*decay + I_t ; s = (v - vth > 0) ; v = v*(1-s)

Sharding: data-parallel over B (B=8 -> 1 batch element per core). The
recurrence is only over T, so no cross-core communication.

Per-core layout: (C,H,W) = (64,64,64) flattened to (128 partitions, 2048),
partition p holds channel c = p//2, so decay/vth are per-partition scalars.

Per step on each core:
  DVE : v  = (w * decay) + x          (scalar_tensor_tensor, fused)
  ACT : g  = Sign(v - vth)            (activation, bias = -vth)
  ACT : s  = Relu(g)                  (exactly 0.0/1.0 spike output)
  DVE : w  = (g <= 0) * v             (scalar_tensor_tensor -> reset)
Input DMA on SyncE (HWDGE), output DMA on ScalarE (HWDGE) so the two
streams live on separate rings and never wait on each other.
"""

import numpy as np
from contextlib import ExitStack

import concourse.bass as bass
import concourse.tile as tile
from concourse import bacc, mybir
from concourse.bass_utils import run_bass_kernel_spmd

T, B, C, H, W = 16, 8, 64, 64, 64
P = 128                 # SBUF partitions
FD = (C * H * W) // P   # free dim per step per core = 2048
NCHUNK = 2              # column chunks for cross-engine pipelining
CH = FD // NCHUNK
N_CORES = 8

_nc_cache = None


def _build_nc():
    nc = bacc.Bacc("TRN2", target_bir_lowering=False, debug=False)
    f32 = mybir.dt.float32
    I_ext = nc.dram_tensor("I", [T, P, FD], f32, kind="ExternalInput").ap()
    decay_ext = nc.dram_tensor("decay", [P, 1], f32, kind="ExternalInput").ap()
    nvth_ext = nc.dram_tensor("nvth", [P, 1], f32, kind="ExternalInput").ap()
    out_ext = nc.dram_tensor("out", [T, P, FD], f32, kind="ExternalOutput").ap()

    with tile.TileContext(nc) as tc, ExitStack() as ctx:
        const_pool = ctx.enter_context(tc.tile_pool(name="const", bufs=1))
        state_pool = ctx.enter_context(tc.tile_pool(name="state", bufs=1))
        x_pool = ctx.enter_context(tc.tile_pool(name="x", bufs=12))
        s_pool = ctx.enter_context(tc.tile_pool(name="s", bufs=6))
        v_pool = ctx.enter_context(tc.tile_pool(name="v", bufs=3))
        g_pool = ctx.enter_context(tc.tile_pool(name="g", bufs=3))

        decay_sb = const_pool.tile([P, 1], f32, tag="decay")
        nvth_sb = const_pool.tile([P, 1], f32, tag="nvth")
        nc.sync.dma_start(out=decay_sb[:], in_=decay_ext[:])
        nc.sync.dma_start(out=nvth_sb[:], in_=nvth_ext[:])

        ws = []
        for chk in range(NCHUNK):
            wt = state_pool.tile([P, CH], f32, tag=f"w{chk}")
            nc.vector.memset(wt[:], 0.0)
            ws.append(wt)

        for t in range(T):
            # Per-chunk input tiles: the chunk-0 recurrence starts as soon as
            # its half arrives instead of waiting for the full 1 MiB step.
            xs = []
            for chk in range(NCHUNK):
                x = x_pool.tile([P, CH], f32, tag="x")
                nc.sync.dma_start(out=x[:], in_=I_ext[t][:, bass.ts(chk, CH)])
                xs.append(x)
            for chk in range(NCHUNK):
                v = v_pool.tile([P, CH], f32, tag="v")
                nc.vector.scalar_tensor_tensor(
                    v[:], ws[chk][:], decay_sb[:], xs[chk][:],
                    op0=mybir.AluOpType.mult, op1=mybir.AluOpType.add,
                )
                g = g_pool.tile([P, CH], f32, tag="g")
                nc.scalar.activation(
                    g[:], v[:], mybir.ActivationFunctionType.Sign,
                    bias=nvth_sb[:],
                )
                s = s_pool.tile([P, CH], f32, tag="s")
                nc.scalar.activation(
                    s[:], g[:], mybir.ActivationFunctionType.Relu,
                )
                nc.vector.scalar_tensor_tensor(
                    ws[chk][:], g[:], 0.0, v[:],
                    op0=mybir.AluOpType.is_le, op1=mybir.AluOpType.mult,
                )
                # Spike store issued from the otherwise-idle GpSimd engine
                # (SWDGE) so neither compute engine pays DMA-issue time.
                nc.gpsimd.dma_start(out=out_ext[t][:, bass.ts(chk, CH)], in_=s[:])

    nc.compile()
    return nc


def get_nc():
    global _nc_cache
    if _nc_cache is None:
        _nc_cache = _build_nc()
    return _nc_cache


def _prep_in_maps(I, tau, vth):
    I = np.ascontiguousarray(np.asarray(I, dtype=np.float32))
    tau = np.asarray(tau, dtype=np.float32)
    vth = np.asarray(vth, dtype=np.float32)
    # Match the reference's broadcast + clamp, in fp32:
    tau_bc = np.broadcast_to(tau, (B, C)) if tau.shape[1] == 1 else tau
    vth_bc = np.broadcast_to(vth, (B, C)) if vth.shape[1] == 1 else vth
    tau_bc = np.maximum(tau_bc, np.float32(0.001))
    vth_bc = np.maximum(vth_bc, np.float32(0.001))
    decay = np.exp(np.float32(-1.0) / tau_bc).astype(np.float32)   # (B, C)

    in_maps = []
    for b in range(B):
        in_maps.append({
            "I": np.ascontiguousarray(I[:, b]).reshape(T, P, FD),
            "decay": np.repeat(decay[b], P // C).reshape(P, 1).astype(np.float32),
            "nvth": np.repeat(-vth_bc[b], P // C).reshape(P, 1).astype(np.float32),
        })
    return in_maps


def run(I, tau, vth, **spmd_kwargs):
    nc = get_nc()
    in_maps = _prep_in_maps(I, tau, vth)
    res = run_bass_kernel_spmd(nc, in_maps, core_ids=list(range(N_CORES)),
                               **spmd_kwargs)
    out = np.stack(
        [res.results[b]["out"].reshape(T, C, H, W) for b in range(B)], axis=1
    ).astype(np.float32)
    return out, res


def kernel(I, tau, vth):
    out, _ = run(I, tau, vth)
    return out


# revision 3
# speedup vs baseline: 1.0558x; 1.0558x over previous
"""AdaptiveLIF spiking-neuron kernel for 8 TRN2 NeuronCores.

Reference recurrence (per element, over T steps):
    v = v*decay + I_t ; s = (v - vth > 0) ; v = v*(1-s)

Sharding: data-parallel over B (B=8 -> 1 batch element per core). The
recurrence is only over T, so no cross-core communication.

Per-core layout: (C,H,W) = (64,64,64) flattened to (128 partitions, 2048),
partition p holds channel c = p//2, so decay/vth are per-partition scalars.

Per step on each core:
  DVE : v  = (w * decay) + x          (scalar_tensor_tensor, fused)
  ACT : g  = Sign(v - vth)            (activation, bias = -vth)
  ACT : s  = Relu(g)                  (exactly 0.0/1.0 spike output)
  DVE : w  = (g <= 0) * v             (scalar_tensor_tensor -> reset)
Input DMA on SyncE (HWDGE), output DMA on ScalarE (HWDGE) so the two
streams live on separate rings and never wait on each other.
"""

import numpy as np
from contextlib import ExitStack

import concourse.bass as bass
import concourse.tile as tile
from concourse import bacc, mybir
from concourse.bass_utils import run_bass_kernel_spmd

T, B, C, H, W = 16, 8, 64, 64, 64
P = 128                 # SBUF partitions
FD = (C * H * W) // P   # free dim per step per core = 2048
NCHUNK = 2              # column chunks for cross-engine pipelining
CH = FD // NCHUNK
N_CORES = 8

_nc_cache = None


def _build_nc():
    nc = bacc.Bacc("TRN2", target_bir_lowering=False, debug=False)
    f32 = mybir.dt.float32
    I_ext = nc.dram_tensor("I", [T, P, FD], f32, kind="ExternalInput").ap()
    decay_ext = nc.dram_tensor("decay", [P, 1], f32, kind="ExternalInput").ap()
    nvth_ext = nc.dram_tensor("nvth", [P, 1], f32, kind="ExternalInput").ap()
    out_ext = nc.dram_tensor("out", [T, P, FD], f32, kind="ExternalOutput").ap()

    with tile.TileContext(nc) as tc, ExitStack() as ctx:
        const_pool = ctx.enter_context(tc.tile_pool(name="const", bufs=1))
        state_pool = ctx.enter_context(tc.tile_pool(name="state", bufs=1))
        x_pool = ctx.enter_context(tc.tile_pool(name="x", bufs=12))
        s_pool = ctx.enter_context(tc.tile_pool(name="s", bufs=6))
        v_pool = ctx.enter_context(tc.tile_pool(name="v", bufs=3))
        g_pool = ctx.enter_context(tc.tile_pool(name="g", bufs=3))

        decay_sb = const_pool.tile([P, 1], f32, tag="decay")
        nvth_sb = const_pool.tile([P, 1], f32, tag="nvth")
        nc.sync.dma_start(out=decay_sb[:], in_=decay_ext[:])
        nc.sync.dma_start(out=nvth_sb[:], in_=nvth_ext[:])

        ws = []
        for chk in range(NCHUNK):
            wt = state_pool.tile([P, CH], f32, tag=f"w{chk}")
            nc.vector.memset(wt[:], 0.0)
            ws.append(wt)

        for t in range(T):
            # Per-chunk input tiles: the chunk-0 recurrence starts as soon as
            # its half arrives instead of waiting for the full 1 MiB step.
            xs = []
            for chk in range(NCHUNK):
                x = x_pool.tile([P, CH], f32, tag="x")
                nc.sync.dma_start(out=x[:], in_=I_ext[t][:, bass.ts(chk, CH)])
                xs.append(x)
            s = s_pool.tile([P, FD], f32, tag="s")
            for chk in range(NCHUNK):
                v = v_pool.tile([P, CH], f32, tag="v")
                nc.vector.scalar_tensor_tensor(
                    v[:], ws[chk][:], decay_sb[:], xs[chk][:],
                    op0=mybir.AluOpType.mult, op1=mybir.AluOpType.add,
                )
                g = g_pool.tile([P, CH], f32, tag="g")
                nc.scalar.activation(
                    g[:], v[:], mybir.ActivationFunctionType.Sign,
                    bias=nvth_sb[:],
                )
                nc.scalar.activation(
                    s[:, bass.ts(chk, CH)], g[:],
                    mybir.ActivationFunctionType.Relu,
                )
                nc.vector.scalar_tensor_tensor(
                    ws[chk][:], g[:], 0.0, v[:],
                    op0=mybir.AluOpType.is_le, op1=mybir.AluOpType.mult,
                )
            # One contiguous 1 MiB store per step, HWDGE on SyncE. Input
            # prefetch runs well ahead, so the FIFO wait on s is harmless.
            nc.sync.dma_start(out=out_ext[t], in_=s[:])

    nc.compile()
    return nc


def get_nc():
    global _nc_cache
    if _nc_cache is None:
        _nc_cache = _build_nc()
    return _nc_cache


def _prep_in_maps(I, tau, vth):
    I = np.ascontiguousarray(np.asarray(I, dtype=np.float32))
    tau = np.asarray(tau, dtype=np.float32)
    vth = np.asarray(vth, dtype=np.float32)
    # Match the reference's broadcast + clamp, in fp32:
    tau_bc = np.broadcast_to(tau, (B, C)) if tau.shape[1] == 1 else tau
    vth_bc = np.broadcast_to(vth, (B, C)) if vth.shape[1] == 1 else vth
    tau_bc = np.maximum(tau_bc, np.float32(0.001))
    vth_bc = np.maximum(vth_bc, np.float32(0.001))
    decay = np.exp(np.float32(-1.0) / tau_bc).astype(np.float32)   # (B, C)

    in_maps = []
    for b in range(B):
        in_maps.append({
            "I": np.ascontiguousarray(I[:, b]).reshape(T, P, FD),
            "decay": np.repeat(decay[b], P // C).reshape(P, 1).astype(np.float32),
            "nvth": np.repeat(-vth_bc[b], P // C).reshape(P, 1).astype(np.float32),
        })
    return in_maps


def run(I, tau, vth, **spmd_kwargs):
    nc = get_nc()
    in_maps = _prep_in_maps(I, tau, vth)
    res = run_bass_kernel_spmd(nc, in_maps, core_ids=list(range(N_CORES)),
                               **spmd_kwargs)
    out = np.stack(
        [res.results[b]["out"].reshape(T, C, H, W) for b in range(B)], axis=1
    ).astype(np.float32)
    return out, res


def kernel(I, tau, vth):
    out, _ = run(I, tau, vth)
    return out


# revision 4
# speedup vs baseline: 1.2524x; 1.1861x over previous
"""AdaptiveLIF spiking-neuron kernel for 8 TRN2 NeuronCores.

Reference recurrence (per element, over T steps):
    v = v*decay + I_t ; s = (v - vth > 0) ; v = v*(1-s)

Sharding: data-parallel over B (B=8 -> 1 batch element per core). The
recurrence is only over T, so no cross-core communication.

Per-core layout: (C,H,W) = (64,64,64) flattened to (128 partitions, 2048),
partition p holds channel c = p//2, so decay/vth are per-partition scalars.

Per step on each core:
  DVE : v  = (w * decay) + x          (scalar_tensor_tensor, fused)
  ACT : g  = Sign(v - vth)            (activation, bias = -vth)
  ACT : s  = Relu(g)                  (exactly 0.0/1.0 spike output)
  DVE : w  = (g <= 0) * v             (scalar_tensor_tensor -> reset)
Input DMA on SyncE (HWDGE), output DMA on ScalarE (HWDGE) so the two
streams live on separate rings and never wait on each other.
"""

import numpy as np
from contextlib import ExitStack

import concourse.bass as bass
import concourse.tile as tile
from concourse import bacc, mybir
from concourse.bass_utils import run_bass_kernel_spmd

T, B, C, H, W = 16, 8, 64, 64, 64
P = 128                 # SBUF partitions
FD = (C * H * W) // P   # free dim per step per core = 2048
NCHUNK = 2              # column chunks for cross-engine pipelining
CH = FD // NCHUNK
N_CORES = 8

_nc_cache = None


def _build_nc():
    nc = bacc.Bacc("TRN2", target_bir_lowering=False, debug=False)
    f32 = mybir.dt.float32
    I_ext = nc.dram_tensor("I", [T, P, FD], f32, kind="ExternalInput").ap()
    decay_ext = nc.dram_tensor("decay", [P, 1], f32, kind="ExternalInput").ap()
    nvth_ext = nc.dram_tensor("nvth", [P, 1], f32, kind="ExternalInput").ap()
    out_ext = nc.dram_tensor("out", [T, P, FD], f32, kind="ExternalOutput").ap()

    with tile.TileContext(nc) as tc, ExitStack() as ctx:
        const_pool = ctx.enter_context(tc.tile_pool(name="const", bufs=1))
        state_pool = ctx.enter_context(tc.tile_pool(name="state", bufs=1))
        x_pool = ctx.enter_context(tc.tile_pool(name="x", bufs=12))
        s_pool = ctx.enter_context(tc.tile_pool(name="s", bufs=6))
        v_pool = ctx.enter_context(tc.tile_pool(name="v", bufs=3))
        g_pool = ctx.enter_context(tc.tile_pool(name="g", bufs=3))

        decay_sb = const_pool.tile([P, 1], f32, tag="decay")
        nvth_sb = const_pool.tile([P, 1], f32, tag="nvth")
        nc.sync.dma_start(out=decay_sb[:], in_=decay_ext[:])
        nc.sync.dma_start(out=nvth_sb[:], in_=nvth_ext[:])

        ws = []
        for chk in range(NCHUNK):
            wt = state_pool.tile([P, CH], f32, tag=f"w{chk}")
            nc.vector.memset(wt[:], 0.0)
            ws.append(wt)

        for t in range(T):
            # Per-chunk input tiles: the chunk-0 recurrence starts as soon as
            # its half arrives instead of waiting for the full 1 MiB step.
            xs = []
            for chk in range(NCHUNK):
                x = x_pool.tile([P, CH], f32, tag="x")
                nc.sync.dma_start(out=x[:], in_=I_ext[t][:, bass.ts(chk, CH)])
                xs.append(x)
            s = s_pool.tile([P, FD], f32, tag="s")
            for chk in range(NCHUNK):
                v = v_pool.tile([P, CH], f32, tag="v")
                nc.vector.scalar_tensor_tensor(
                    v[:], ws[chk][:], decay_sb[:], xs[chk][:],
                    op0=mybir.AluOpType.mult, op1=mybir.AluOpType.add,
                )
                g = g_pool.tile([P, CH], f32, tag="g")
                nc.scalar.activation(
                    g[:], v[:], mybir.ActivationFunctionType.Sign,
                    bias=nvth_sb[:],
                )
                nc.scalar.activation(
                    s[:, bass.ts(chk, CH)], g[:],
                    mybir.ActivationFunctionType.Relu,
                )
                nc.vector.scalar_tensor_tensor(
                    ws[chk][:], g[:], 0.0, v[:],
                    op0=mybir.AluOpType.is_le, op1=mybir.AluOpType.mult,
                )
            # One contiguous 1 MiB store per step. Mid-kernel stores go via
            # GpSimd (SWDGE): a separate issue path, so the SyncE FIFO keeps
            # streaming input prefetch at full rate. The last steps use
            # ScalarE's HWDGE ring instead — SWDGE's Q7 descriptor queue
            # drains slowly and would stretch the kernel tail.
            if t < T - 2:
                nc.gpsimd.dma_start(out=out_ext[t], in_=s[:])
            else:
                nc.scalar.dma_start(out=out_ext[t], in_=s[:])

    nc.compile()
    return nc


def get_nc():
    global _nc_cache
    if _nc_cache is None:
        _nc_cache = _build_nc()
    return _nc_cache


def _prep_in_maps(I, tau, vth):
    I = np.ascontiguousarray(np.asarray(I, dtype=np.float32))
    tau = np.asarray(tau, dtype=np.float32)
    vth = np.asarray(vth, dtype=np.float32)
    # Match the reference's broadcast + clamp, in fp32:
    tau_bc = np.broadcast_to(tau, (B, C)) if tau.shape[1] == 1 else tau
    vth_bc = np.broadcast_to(vth, (B, C)) if vth.shape[1] == 1 else vth
    tau_bc = np.maximum(tau_bc, np.float32(0.001))
    vth_bc = np.maximum(vth_bc, np.float32(0.001))
    decay = np.exp(np.float32(-1.0) / tau_bc).astype(np.float32)   # (B, C)

    in_maps = []
    for b in range(B):
        in_maps.append({
            "I": np.ascontiguousarray(I[:, b]).reshape(T, P, FD),
            "decay": np.repeat(decay[b], P // C).reshape(P, 1).astype(np.float32),
            "nvth": np.repeat(-vth_bc[b], P // C).reshape(P, 1).astype(np.float32),
        })
    return in_maps


def run(I, tau, vth, **spmd_kwargs):
    nc = get_nc()
    in_maps = _prep_in_maps(I, tau, vth)
    res = run_bass_kernel_spmd(nc, in_maps, core_ids=list(range(N_CORES)),
                               **spmd_kwargs)
    out = np.stack(
        [res.results[b]["out"].reshape(T, C, H, W) for b in range(B)], axis=1
    ).astype(np.float32)
    return out, res


def kernel(I, tau, vth):
    out, _ = run(I, tau, vth)
    return out
